# revision 51
# baseline (speedup 1.0000x reference)
"""MLA attention kernel for Trainium2 (8 NeuronCores, Bass/Tile).

Problem (nn_MLAAttention): B=2, S=2048, D=2048, 16 heads x 128, latent 512,
RoPE, causal softmax, output projection.

Sharding: core c handles batch b=c//4 and 4 heads hg=c%4 (tensor parallel over
heads, data parallel over batch). Each core computes a partial output
(attn-out of its heads through its Wo column-slice); the host sums partials
within a batch and transposes back.

On-core dataflow (all activations kept feature-major, "transposed"):
  XT[d,s] --Wq/Wc--> qT[dh,s] (+RoPE), c_kvT[dl,s]
  c_kvT --Wk--> kT[dh,s] (+RoPE);  c_kvT --Wv--> v[s,hf] (natural)
  scoresT[sk,sq] = kT.T-slice @ qT-slice  (per head, causal tiles only)
  expT = exp(scores/sqrt(dh)) * causal_mask   (bf16; mask-mult on DVE 2x)
  exacc[sk,sq] = sum_j expT_j   (DVE 2x-mode adds, one acc per block)
  sums_bcast[sk,sq] = ones[128,128].T @ exacc  (one PE matmul both reduces
      over partitions and broadcasts the result to all 128 rows)
  rb = reciprocal_approx_fast(sums_bcast)  (single custom-DVE op)
  oT~[dh,sq] = v-slice.T @ expT  (unnormalized)
  oT = oT~ * rb  (DVE; PE never waits on the reciprocal chain)
  outT[dout,s] = WoT.T-slices @ oT  (partial output projection)

Schedule: phase A (q+ckv projection, 8-bank PSUM rotation, q-subloop before
ckv-subloop so each quarter's RoPE DVE chain drains under the ckv k-loop),
then a single software-pipelined region where kT/v decompression groups are
interleaved between attention blocks (kt for head h+1 and v chunks are
produced while head h's scores/softmax/PV run), then the output projection
with PSUM evacuation split across ACT/DVE and DMA issued from the idle SP
queue. Diagonal score chunks are narrowed to the live query range (512/384/
256/128), 12/16 WoT chunks preload during attention, and the first 12
output-projection blocks (s-tile 0) are emitted inside head 3's attention
to fill its chain bubbles. gpsimd runs NO custom Pool ops (library swaps
around partition_all_reduce serialized the old softmax and stalled PE
12-22us per block); the whole normalize is PE-matmul + 2 cheap DVE ops.
A ckv AllGather sharding variant (each core compresses S/4, groups
[[0-3],[4-7]] gather via DRAM bounce) was measured SLOWER (353us vs 336us):
the 4-core AllGather takes ~50us wall and gates the attention start, eating
the 73us of saved PE time. See kernel_v5_cc.py.

All matmul operands (x, weights, c_kv, qt, kt, v, exp) are bf16: full PE
rate, cheap LDWEIGHTS, half the DMA/SBUF of f32. The softmax denominator
costs one DVE add chain + one PE ones-matmul column-sum + tiny reciprocal +
PE broadcast per block (no gpsimd custom ops, which would stall PE behind
Pool library swaps).
"""

import sys

if "/opt/trn_rl_repo" not in sys.path:
    sys.path.insert(0, "/opt/trn_rl_repo")

import math
import numpy as np

import concourse.bass  # noqa: F401  (registers bass types used by tile)
import concourse.tile as tile
from concourse import bacc, mybir
from concourse.bass_utils import run_bass_kernel_spmd

F32 = mybir.dt.float32
F32R = mybir.dt.float32r
BF16 = mybir.dt.bfloat16
FP8 = mybir.dt.float8e4
EXP = mybir.ActivationFunctionType.Exp

B, S, D, DL, H, DH = 2, 2048, 2048, 512, 16, 128
HL = 4  # heads per core
HF = HL * DH  # 512 local head-features
NCORES = 8
SCALE = 1.0 / math.sqrt(DH)

_CACHED = {}


def _build():
    nc = bacc.Bacc("TRN2", target_bir_lowering=False, debug=False, num_devices=8)

    xt_d = nc.dram_tensor("xt", [D, S], BF16, kind="ExternalInput")
    wqct_d = nc.dram_tensor("wqct", [128, 16, 1024], BF16, kind="ExternalInput")
    wkvt_d = nc.dram_tensor("wkvt", [128, 4, 1024], BF16, kind="ExternalInput")
    wot_d = nc.dram_tensor("wot", [128, 16, 512], BF16, kind="ExternalInput")
    cos_d = nc.dram_tensor("cost", [128, S], F32, kind="ExternalInput")
    sin_d = nc.dram_tensor("sint", [128, S], F32, kind="ExternalInput")
    mask_d = nc.dram_tensor("masks", [128, 1280], BF16, kind="ExternalInput")
    onesk_d = nc.dram_tensor("onesk", [128, 128], BF16, kind="ExternalInput")
    out_d = nc.dram_tensor("outt", [D, S], F32, kind="ExternalOutput")

    with tile.TileContext(nc) as tc:
        _body(
            nc, tc, xt_d, wqct_d, wkvt_d, wot_d, cos_d, sin_d, mask_d,
            onesk_d, out_d,
        )
    nc.compile()
    return nc


def _rope_evac(nc, tmp_pool, psum, out_ap, cos_ap, sinm_ap):
    """out = psum*cos + rotate_half(psum)*sinm, written as float32r.

    sinm is sign-folded (-sin on partitions 0:64, +sin on 64:128) so the
    combine is a single full-width add.
    """
    t1 = tmp_pool.tile([128, 512], F32, tag="rope1")
    nc.vector.tensor_mul(t1[:], psum[:], cos_ap)
    qr = tmp_pool.tile([128, 512], F32, tag="roper")
    nc.vector.tensor_mul(qr[0:64, :], psum[64:128, :], sinm_ap[0:64, :])
    nc.vector.tensor_mul(qr[64:128, :], psum[0:64, :], sinm_ap[64:128, :])
    nc.vector.tensor_add(out_ap[:], t1[:], qr[:])


def _body(
    nc, tc, xt_d, wqct_d, wkvt_d, wot_d, cos_d, sin_d, mask_d,
    onesk_d, out_d,
):
    import contextlib

    with contextlib.ExitStack() as ctx:
        ent = ctx.enter_context

        # ---- persistent pools (right heap side; phase pools use left) ----
        p_qt = ent(tc.tile_pool(name="qt", bufs=1, side="right"))
        p_small = ent(tc.tile_pool(name="small", bufs=1, side="right"))

        qt = p_qt.tile([128, HL, S], BF16, tag="qt")  # [dh, h, s]
        masks = p_small.tile([128, 1280], BF16, tag="masks")
        # all-ones stationary for the softmax-denominator broadcast-sum
        ones_k = p_small.tile([128, 128], BF16, tag="ones_k")
        nc.sync.dma_start(ones_k[:], onesk_d[:])

        # scoped pools that die before the output projection
        p_wB = tc.alloc_tile_pool(name="wB", bufs=1)
        p_ck = tc.alloc_tile_pool(name="ck", bufs=1)
        p_tmp = tc.alloc_tile_pool(name="tmp", bufs=6)

        ckv = p_ck.tile([128, 4, S], BF16, tag="ckv")  # [dl%128, dl//128, s]
        cost = p_ck.tile([128, S], F32, tag="cos")
        sint = p_ck.tile([128, S], F32, tag="sin")
        wkvt = p_wB.tile([128, 4, 1024], BF16, tag="wkvt")

        # ============ Phase A: qT + c_kvT projection ============
        # 4+4 PSUM bank rotation. Per s-quarter: q groups then c_kv groups;
        # each sub-loop's evacuation overlaps the other's matmuls.
        with (
            tc.tile_pool(name="wA", bufs=1) as p_wA,
            tc.tile_pool(name="xts", bufs=12) as p_xt,
            tc.tile_pool(name="psAq", bufs=4, space="PSUM") as ps_aq,
            tc.tile_pool(name="psAc", bufs=4, space="PSUM") as ps_ac,
        ):
            wqct = p_wA.tile([128, 16, 1024], BF16, tag="wA")
            # first weight chunks lead the queue: the very first LDWEIGHTS
            # waits only on wqct k=0, so everything else goes after it
            for k in range(4):
                nc.sync.dma_start(wqct[:, k, :], wqct_d[:, k, :])
            nc.sync.dma_start(masks[:], mask_d[:])
            for k in range(4, 16):
                nc.sync.dma_start(wqct[:, k, :], wqct_d[:, k, :])
            for i in range(4):
                nc.sync.dma_start(wkvt[:, i, :], wkvt_d[:, i, :])

            def ckv_subloop(q, sq):
                caccs = [
                    ps_ac.tile([128, 512], F32, tag="pac", name=f"pac{m}")
                    for m in range(4)
                ]
                for k in range(16):
                    xt = p_xt.tile([128, 512], BF16, tag="xt")
                    nc.scalar.dma_start(xt[:], xt_d[k * 128 : (k + 1) * 128, sq])
                    for i in range(4):
                        nc.tensor.matmul(
                            caccs[i][:],
                            wqct[:, k, 512 + i * 128 : 640 + i * 128],
                            xt[:],
                            start=(k == 0),
                            stop=(k == 15),
                        )
                for i in range(4):
                    nc.scalar.copy(ckv[:, i, sq], caccs[i][:])

            def q_subloop(q, sq):
                qaccs = [
                    ps_aq.tile([128, 512], F32, tag="paq", name=f"pq{q}{m}")
                    for m in range(4)
                ]
                for k in range(16):
                    xt = p_xt.tile([128, 512], BF16, tag="xt")
                    nc.gpsimd.dma_start(xt[:], xt_d[k * 128 : (k + 1) * 128, sq])
                    for i in range(4):
                        nc.tensor.matmul(
                            qaccs[i][:],
                            wqct[:, k, i * 128 : (i + 1) * 128],
                            xt[:],
                            start=(k == 0),
                            stop=(k == 15),
                        )
                for i in range(4):
                    _rope_evac(
                        nc,
                        p_tmp,
                        qaccs[i][:],
                        qt[:, i, sq],
                        cost[:, sq],
                        sint[:, sq],
                    )

            for q in range(4):  # s-quarter
                sq = slice(q * 512, (q + 1) * 512)
                nc.sync.dma_start(cost[:, sq], cos_d[:, sq])
                nc.sync.dma_start(sint[:, sq], sin_d[:, sq])
                q_subloop(q, sq)
                ckv_subloop(q, sq)

        # ============ Merged B+C: kT/v decompression interleaved with
        # attention (scoresT / softmax / PV) ============
        p_kv = ent(tc.tile_pool(name="kv", bufs=1, side="right"))
        kt = p_kv.tile([128, HL, S], BF16, tag="kt")  # [dh, h, s]
        v = p_kv.tile([128, 16, HF], BF16, tag="v")  # [s%128, s//128, hf]
        # fp8 shadow of v for the off-diagonal DoubleRow PV path
        v8 = p_kv.tile([128, 16, HF], FP8, tag="v8")

        # first half of WoT loads during the attention region (fresh SBUF,
        # no WAR wait) so the output projection starts without a weight stall
        p_wDe = ent(tc.tile_pool(name="wDe", bufs=1, side="right"))
        wde = p_wDe.tile([128, 12, 512], BF16, tag="wde")
        for m in range(12):
            nc.sync.dma_start(wde[:, m, :], wot_d[:, m, :])

        p_ex = tc.alloc_tile_pool(name="ex", bufs=8)
        p_acc = tc.alloc_tile_pool(name="exacc", bufs=4)
        p_bc = tc.alloc_tile_pool(name="bc", bufs=2)
        p_osb = tc.alloc_tile_pool(name="osb", bufs=2)
        p_oei = tc.alloc_tile_pool(name="oei", bufs=2)
        ps_bkv = tc.alloc_tile_pool(name="psBKV", bufs=1, space="PSUM")
        ps_s = tc.alloc_tile_pool(name="psS", bufs=2, space="PSUM")
        ps_o = tc.alloc_tile_pool(name="psO", bufs=2, space="PSUM")
        ps_n = tc.alloc_tile_pool(name="psN", bufs=1, space="PSUM")

        def kt_group(g):
            m, n = divmod(g, 4)
            sn = slice(n * 512, (n + 1) * 512)
            acc = ps_bkv.tile([128, 512], F32, tag="pkv", name=f"pk{g}")
            for i in range(4):
                nc.tensor.matmul(
                    acc[:],
                    wkvt[:, i, m * 128 : (m + 1) * 128],
                    ckv[:, i, sn],
                    start=(i == 0),
                    stop=(i == 3),
                )
            _rope_evac(nc, p_tmp, acc[:], kt[:, m, sn], cost[:, sn], sint[:, sn])

        def v_group(g):
            acc2 = ps_bkv.tile([128, 512], F32, tag="pkv", name=f"pv{g}")
            for i in range(4):
                nc.tensor.matmul(
                    acc2[:],
                    ckv[:, i, g * 128 : (g + 1) * 128],
                    wkvt[:, i, 512:1024],
                    start=(i == 0),
                    stop=(i == 3),
                )
            nc.scalar.copy(v[:, g, :], acc2[:])
            nc.vector.tensor_copy(v8[:, g, :], acc2[:])

        # decompression work interleaved after attention block (h, t); kt
        # group g feeds block (g//4, g%4), v group g feeds PV column j=g.
        interleave = {
            (0, 0): [("kt", 1), ("v", 4), ("v", 5), ("v", 6), ("v", 7)],
            (0, 1): [("kt", 2), ("v", 8), ("v", 9), ("v", 10), ("v", 11)],
            (0, 2): [("kt", 3), ("v", 12), ("v", 13), ("v", 14), ("v", 15)],
            (0, 3): [("kt", 4), ("kt", 5)],
            (1, 0): [("kt", 6)],
            (1, 1): [("kt", 7)],
            (1, 2): [("kt", 8)],
            (1, 3): [("kt", 9)],
            (2, 0): [("kt", 10)],
            (2, 1): [("kt", 11)],
            (2, 2): [("kt", 12), ("kt", 13)],
            (2, 3): [("kt", 14), ("kt", 15)],
        }

        def v_group_ps(g):
            acc2 = ps_s.tile([128, 1024], F32, tag="s", name=f"pvs{g}")
            for i in range(4):
                nc.tensor.matmul(
                    acc2[:, 0:512],
                    ckv[:, i, g * 128 : (g + 1) * 128],
                    wkvt[:, i, 512:1024],
                    start=(i == 0),
                    stop=(i == 3),
                )
            nc.vector.tensor_copy(v[:, g, :], acc2[:, 0:512])
            nc.scalar.copy(v8[:, g, :], acc2[:, 0:512])

        # prologue: first key block of head 0 + v chunks for its PV
        kt_group(0)
        v_group(0)
        v_group_ps(1)
        v_group_ps(2)
        v_group(3)

        # One continuous software pipeline across all (h, t) blocks:
        # scores/exp/mask run 2 pairs ahead of sums/pv, including over
        # block boundaries, so PE never drains waiting on ACT+DVE.
        blocks = [(h, t) for h in range(HL) for t in range(4)]
        ctxs = {}
        pend = []

        def emit_sums_pv(bi, jj, ex, lay):
            h, t, acc_o, exacc = ctxs[bi]
            nj = 4 * (t + 1)
            if lay == "fp8":
                # off-diagonal pair: bf16 accumulator absorbs the fp8 exp
                # tiles; one DoubleRow matmul covers both key chunks
                if jj == 0:
                    nc.vector.tensor_copy(exacc[:], ex[:, 0, :])
                else:
                    nc.vector.tensor_add(exacc[:], exacc[:], ex[:, 0, :])
                nc.vector.tensor_add(exacc[:], exacc[:], ex[:, 1, :])
                nc.tensor.matmul(
                    acc_o[:],
                    v8[:, jj : jj + 2, h * 128 : (h + 1) * 128],
                    ex[:, 0:2, :],
                    start=(jj == 0),
                    stop=False,
                    perf_mode=mybir.MatmulPerfMode.DoubleRow,
                )
                return
            for j, cs, w, qo in lay:
                exsl = ex[:, cs : cs + w]
                # softmax denominator: accumulate exp tiles on DVE (bf16
                # packed = 2x mode); the partition reduce happens once per
                # block in a single 512-row ones-matmul.
                if j == 0:
                    nc.vector.tensor_copy(exacc[:], exsl)
                else:
                    nc.vector.tensor_add(
                        exacc[:, qo:512], exacc[:, qo:512], exsl
                    )
                nc.tensor.matmul(
                    acc_o[:, qo:512],
                    v[:, j, h * 128 : (h + 1) * 128],
                    exsl,
                    start=(j == 0), stop=(j == nj - 1),
                )
            if jj == nj - 2:
                _normalize(bi)

        def d_block_early(m):
            # output projection (n=0 s-tile) emitted inside head 3; qt
            # [:, :, 0:512] is final once every head's t=0 block normalized
            acc = ps_bkv.tile([128, 512], F32, tag="pkv", name=f"pde{m}")
            for i in range(4):
                nc.tensor.matmul(
                    acc[:],
                    wde[:, m, i * 128 : (i + 1) * 128],
                    qt[:, i, 0:512],
                    start=(i == 0),
                    stop=(i == 3),
                )
            o = p_oei.tile([128, 512], F32, tag="oei", name=f"oei{m}")
            if m % 2 == 0:
                nc.scalar.copy(o[:], acc[:])
            else:
                nc.vector.tensor_copy(o[:], acc[:])
            nc.sync.dma_start(out_d[m * 128 : (m + 1) * 128, 0:512], o[:])

        def _normalize(bi):
            h, t, acc_o, exacc = ctxs.pop(bi)
            sq = slice(t * 512, (t + 1) * 512)
            ou = p_osb.tile([128, 512], F32, tag="ou", name=f"ou{bi}")
            nc.scalar.copy(ou[:], acc_o[:])
            # softmax denominator: all-ones matmul broadcast-sums exacc
            # across partitions into a scratch bank (PE depends only on the
            # last exacc add), then a fast approx reciprocal + multiply on
            # DVE finish the normalize off the PE critical path.
            nb = ps_n.tile([128, 512], F32, tag="nb", name=f"nb{bi}")
            nc.tensor.matmul(nb[:, :], ones_k[:], exacc[:], start=True, stop=True)
            rbb = p_bc.tile([128, 512], F32, tag="rb", name=f"rb{bi}")
            nc.vector.reciprocal_approx_fast(rbb[:], nb[:, :])
            nc.vector.tensor_mul(qt[:, h, sq], ou[:], rbb[:])

        def chunk_width(di):
            # diagonal chunks only compute the live (unmasked) query range:
            # chunk di covers keys >= 128*di of its quarter, so queries below
            # 128*di are fully masked and skipped (bf16 matmuls are full-rate
            # at any width).
            if di < 1:
                return 512
            return (384, 256, 128)[di - 1]

        for bi, (h, t) in enumerate(blocks):
            nj = 4 * (t + 1)
            ctxs[bi] = (
                h,
                t,
                ps_o.tile([128, 512], F32, tag="o", name=f"o{bi}"),
                p_acc.tile([128, 512], BF16, tag="exacc", name=f"exacc{bi}"),
            )
            for jj in range(0, nj, 2):
                di0 = jj - 4 * t
                if di0 < 0:
                    # both chunks off-diagonal: fp8 exp + DoubleRow PV
                    ps = ps_s.tile([128, 1024], F32, tag="s", name=f"s{bi}_{jj}")
                    for d_ in range(2):
                        j = jj + d_
                        nc.tensor.matmul(
                            ps[:, d_ * 512 : (d_ + 1) * 512],
                            kt[:, h, j * 128 : (j + 1) * 128],
                            qt[:, h, t * 512 : (t + 1) * 512],
                            start=True,
                            stop=True,
                        )
                    ex8 = p_ex.tile(
                        [128, 2, 512], FP8, tag="ex8", name=f"e8{bi}_{jj}"
                    )
                    nc.scalar.activation(ex8[:, 0:2, :], ps[:], EXP, scale=SCALE)
                    pend.append((bi, jj, ex8, "fp8"))
                    if len(pend) > 2:
                        emit_sums_pv(*pend.pop(0))
                    continue
                if di0 == 2:
                    # narrow diag pair (256+128 cols) fits the normalize
                    # bank, which idles between per-block normalizes
                    ps = ps_n.tile([128, 512], F32, tag="nb", name=f"sd{bi}")
                else:
                    ps = ps_s.tile([128, 1024], F32, tag="s", name=f"s{bi}_{jj}")
                lay = []
                cs = 0
                for d_ in range(2):
                    j = jj + d_
                    w = chunk_width(di0 + d_)
                    qo = 512 - w
                    nc.tensor.matmul(
                        ps[:, cs : cs + w],
                        kt[:, h, j * 128 : (j + 1) * 128],
                        qt[:, h, t * 512 + qo : (t + 1) * 512],
                        start=True,
                        stop=True,
                    )
                    lay.append((j, cs, w, qo))
                    cs += w
                ex = p_ex.tile([128, 1024], BF16, tag="ex", name=f"ex{bi}_{jj}")
                nc.scalar.activation(ex[:, :cs], ps[:, :cs], EXP, scale=SCALE)
                if di0 >= 0:  # diagonal pair -> causal mask (DVE, bf16 2x)
                    mb = 0 if di0 == 0 else 896
                    nc.vector.tensor_mul(
                        ex[:, :cs], ex[:, :cs], masks[:, mb : mb + cs]
                    )
                pend.append((bi, jj, ex, lay))
                if len(pend) > 2:
                    emit_sums_pv(*pend.pop(0))
            for kind, g in interleave.get((h, t), []):
                kt_group(g) if kind == "kt" else v_group(g)
            if (h, t) in ((3, 1), (3, 2), (3, 3)):
                for m in range(4 * (t - 1), 4 * t):
                    d_block_early(m)
        while pend:
            emit_sums_pv(*pend.pop(0))

        # free decompression-scope SBUF (LIFO), then preload WoT
        ps_n.release()
        ps_o.release()
        ps_s.release()
        ps_bkv.release()
        p_oei.release()
        p_osb.release()
        p_bc.release()
        p_acc.release()
        p_ex.release()
        p_tmp.release()
        p_ck.release()
        p_wB.release()

        # ================= Phase D: output projection =================
        # n-outer so the first s-tile only waits on the last head's t=0
        # normalize; PSUM evacuation alternates ACT/DVE; DMA from SP queue.
        with (
            tc.tile_pool(name="wD", bufs=1) as p_wD,
            tc.tile_pool(name="oev", bufs=6) as p_oev,
            tc.tile_pool(name="psD", bufs=4, space="PSUM") as ps_d,
        ):
            wot = p_wD.tile([128, 4, 512], BF16, tag="wot")
            for m in range(4):
                nc.sync.dma_start(wot[:, m, :], wot_d[:, m + 12, :])
            for n in range(4):  # s tile
                sn = slice(n * 512, (n + 1) * 512)
                for m in range(16):  # dout chunk
                    if n == 0 and m < 12:
                        continue  # emitted early inside head 3
                    acc = ps_d.tile([128, 512], F32, tag="pd")
                    wsrc = wde[:, m, :] if m < 12 else wot[:, m - 12, :]
                    for i in range(4):
                        nc.tensor.matmul(
                            acc[:],
                            wsrc[:, i * 128 : (i + 1) * 128],
                            qt[:, i, sn],
                            start=(i == 0),
                            stop=(i == 3),
                        )
                    o = p_oev.tile([128, 512], F32, tag="oev")
                    if m % 2 == 0:
                        nc.scalar.copy(o[:], acc[:])
                    else:
                        nc.vector.tensor_copy(o[:], acc[:])
                    nc.sync.dma_start(out_d[m * 128 : (m + 1) * 128, sn], o[:])


def _rope_tables():
    inv = 1.0 / (10000.0 ** (np.arange(0, DH, 2, dtype=np.float32) / DH))
    t = np.arange(S, dtype=np.float32)
    freqs = np.outer(t, inv)  # (S, 64)
    emb = np.concatenate([freqs, freqs], axis=-1)  # (S, 128)
    sinm = np.sin(emb).astype(np.float32)
    sinm[:, :64] *= -1.0  # sign-folded for the rotate-half add
    return (
        np.ascontiguousarray(np.cos(emb).T.astype(np.float32)),
        np.ascontiguousarray(sinm.T),
    )


def _masks():
    p = np.arange(128)[:, None]
    segs = []
    for di, w in [(0, 512), (1, 384), (2, 256), (3, 128)]:
        q = np.arange(512 - w, 512)[None, :]
        segs.append((p <= q - 128 * di).astype(np.float32))
    return np.ascontiguousarray(np.concatenate(segs, axis=1))


def kernel(hidden_states, Wq, Wc, Wk, Wv, Wo, _trace=False):
    hidden_states = np.asarray(hidden_states, dtype=np.float32)
    Wq = np.asarray(Wq, dtype=np.float32)
    Wc = np.asarray(Wc, dtype=np.float32)
    Wk = np.asarray(Wk, dtype=np.float32)
    Wv = np.asarray(Wv, dtype=np.float32)
    Wo = np.asarray(Wo, dtype=np.float32)

    if "nc" not in _CACHED:
        _CACHED["nc"] = _build()
    nc = _CACHED["nc"]

    import ml_dtypes

    bf16 = ml_dtypes.bfloat16
    cost, sint = _rope_tables()
    masks = _masks().astype(bf16)
    wct = np.ascontiguousarray(Wc.T)  # (D, DL)
    xts = [np.ascontiguousarray(hidden_states[b].T).astype(bf16) for b in range(B)]

    in_maps = []
    for c in range(NCORES):
        b, hg = divmod(c, HL)
        hs = hg * HF
        wq = Wq[hs : hs + HF].T  # (D, HF)
        wqct = np.concatenate([wq, wct], axis=1)  # (D, 1024)
        wqct = np.ascontiguousarray(wqct.reshape(16, 128, 1024).transpose(1, 0, 2)).astype(bf16)
        wk = Wk[hs : hs + HF].T  # (DL, HF)
        wv = Wv[hs : hs + HF].T
        wkvt = np.concatenate([wk, wv], axis=1)  # (DL, 1024)
        wkvt = np.ascontiguousarray(wkvt.reshape(4, 128, 1024).transpose(1, 0, 2)).astype(bf16)
        wot = Wo[:, hs : hs + HF].T  # (HF, D)
        wot = np.ascontiguousarray(
            wot.reshape(4, 128, 16, 128).transpose(1, 2, 0, 3).reshape(128, 16, 512)
        ).astype(bf16)
        in_maps.append(
            {
                "xt": xts[b],
                "wqct": wqct,
                "wkvt": wkvt,
                "wot": wot,
                "cost": cost,
                "sint": sint,
                "masks": masks,
                "onesk": np.ones((128, 128), bf16),
            }
        )

    res = run_bass_kernel_spmd(
        nc, in_maps, core_ids=list(range(NCORES)), trace=_trace
    )
    _CACHED["last_results"] = res

    out = np.empty((B, S, D), np.float32)
    for b in range(B):
        acc = res.results[4 * b]["outt"].astype(np.float32)
        for c in range(4 * b + 1, 4 * b + 4):
            acc = acc + res.results[c]["outt"]
        out[b] = acc.T
    return out



# revision 52
# speedup vs baseline: 1.0584x; 1.0584x over previous
"""MLA attention kernel for Trainium2 (8 NeuronCores, Bass/Tile).

Problem (nn_MLAAttention): B=2, S=2048, D=2048, 16 heads x 128, latent 512,
RoPE, causal softmax, output projection.

Sharding: core c handles batch b=c//4 and 4 heads hg=c%4 (tensor parallel over
heads, data parallel over batch). Each core computes a partial output
(attn-out of its heads through its Wo column-slice); the host sums partials
within a batch and transposes back.

On-core dataflow (all activations kept feature-major, "transposed"):
  XT[d,s] --Wq/Wc--> qT[dh,s] (+RoPE), c_kvT[dl,s]
  c_kvT --Wk--> kT[dh,s] (+RoPE);  c_kvT --Wv--> v[s,hf] (natural)
  scoresT[sk,sq] = kT.T-slice @ qT-slice  (per head, causal tiles only)
  expT = exp(scores/sqrt(dh)) * causal_mask   (bf16; mask-mult on DVE 2x)
  exacc[sk,sq] = sum_j expT_j   (DVE 2x-mode adds, one acc per block)
  sums_bcast[sk,sq] = ones[128,128].T @ exacc  (one PE matmul both reduces
      over partitions and broadcasts the result to all 128 rows)
  rb = reciprocal_approx_fast(sums_bcast)  (single custom-DVE op)
  oT~[dh,sq] = v-slice.T @ expT  (unnormalized)
  oT = oT~ * rb  (DVE; PE never waits on the reciprocal chain)
  outT[dout,s] = WoT.T-slices @ oT  (partial output projection)

Schedule: phase A (q+ckv projection, 8-bank PSUM rotation, q-subloop before
ckv-subloop so each quarter's RoPE DVE chain drains under the ckv k-loop),
then a single software-pipelined region where kT/v decompression groups are
interleaved between attention blocks (kt for head h+1 and v chunks are
produced while head h's scores/softmax/PV run), then the output projection
with PSUM evacuation split across ACT/DVE and DMA issued from the idle SP
queue. Diagonal score chunks are narrowed to the live query range (512/384/
256/128), 12/16 WoT chunks preload during attention, and the first 12
output-projection blocks (s-tile 0) are emitted inside head 3's attention
to fill its chain bubbles. gpsimd runs NO custom Pool ops (library swaps
around partition_all_reduce serialized the old softmax and stalled PE
12-22us per block); the whole normalize is PE-matmul + 2 cheap DVE ops.
A ckv AllGather sharding variant (each core compresses S/4, groups
[[0-3],[4-7]] gather via DRAM bounce) was measured SLOWER (353us vs 336us):
the 4-core AllGather takes ~50us wall and gates the attention start, eating
the 73us of saved PE time. See kernel_v5_cc.py.

All matmul operands (x, weights, c_kv, qt, kt, v, exp) are bf16: full PE
rate, cheap LDWEIGHTS, half the DMA/SBUF of f32. The softmax denominator
costs one DVE add chain + one PE ones-matmul column-sum + tiny reciprocal +
PE broadcast per block (no gpsimd custom ops, which would stall PE behind
Pool library swaps).
"""

import sys

if "/opt/trn_rl_repo" not in sys.path:
    sys.path.insert(0, "/opt/trn_rl_repo")

import math
import numpy as np

import concourse.bass  # noqa: F401  (registers bass types used by tile)
import concourse.tile as tile
from concourse import bacc, mybir
from concourse.bass_utils import run_bass_kernel_spmd

F32 = mybir.dt.float32
F32R = mybir.dt.float32r
BF16 = mybir.dt.bfloat16
EXP = mybir.ActivationFunctionType.Exp

B, S, D, DL, H, DH = 2, 2048, 2048, 512, 16, 128
HL = 4  # heads per core
HF = HL * DH  # 512 local head-features
NCORES = 8
SCALE = 1.0 / math.sqrt(DH)

_CACHED = {}


def _build():
    nc = bacc.Bacc("TRN2", target_bir_lowering=False, debug=False, num_devices=8)

    xt_d = nc.dram_tensor("xt", [D, S], BF16, kind="ExternalInput")
    wqct_d = nc.dram_tensor("wqct", [128, 16, 1024], BF16, kind="ExternalInput")
    wkvt_d = nc.dram_tensor("wkvt", [128, 4, 1024], BF16, kind="ExternalInput")
    wot_d = nc.dram_tensor("wot", [128, 16, 512], BF16, kind="ExternalInput")
    cos_d = nc.dram_tensor("cost", [128, S], F32, kind="ExternalInput")
    sin_d = nc.dram_tensor("sint", [128, S], F32, kind="ExternalInput")
    mask_d = nc.dram_tensor("masks", [128, 1280], BF16, kind="ExternalInput")
    onesk_d = nc.dram_tensor("onesk", [128, 128], BF16, kind="ExternalInput")
    out_d = nc.dram_tensor("outt", [D, S], F32, kind="ExternalOutput")

    with tile.TileContext(nc) as tc:
        _body(
            nc, tc, xt_d, wqct_d, wkvt_d, wot_d, cos_d, sin_d, mask_d,
            onesk_d, out_d,
        )
    nc.compile()
    return nc


def _rope_evac(nc, tmp_pool, psum, out_ap, cos_ap, sinm_ap):
    """out = psum*cos + rotate_half(psum)*sinm, written as float32r.

    sinm is sign-folded (-sin on partitions 0:64, +sin on 64:128) so the
    combine is a single full-width add.
    """
    t1 = tmp_pool.tile([128, 512], F32, tag="rope1")
    nc.vector.tensor_mul(t1[:], psum[:], cos_ap)
    qr = tmp_pool.tile([128, 512], F32, tag="roper")
    nc.vector.tensor_mul(qr[0:64, :], psum[64:128, :], sinm_ap[0:64, :])
    nc.vector.tensor_mul(qr[64:128, :], psum[0:64, :], sinm_ap[64:128, :])
    nc.vector.tensor_add(out_ap[:], t1[:], qr[:])


def _body(
    nc, tc, xt_d, wqct_d, wkvt_d, wot_d, cos_d, sin_d, mask_d,
    onesk_d, out_d,
):
    import contextlib

    with contextlib.ExitStack() as ctx:
        ent = ctx.enter_context

        # ---- persistent pools (right heap side; phase pools use left) ----
        p_qt = ent(tc.tile_pool(name="qt", bufs=1, side="right"))
        p_small = ent(tc.tile_pool(name="small", bufs=1, side="right"))

        qt = p_qt.tile([128, HL, S], BF16, tag="qt")  # [dh, h, s]
        masks = p_small.tile([128, 1280], BF16, tag="masks")
        # all-ones stationary for the softmax-denominator broadcast-sum
        ones_k = p_small.tile([128, 128], BF16, tag="ones_k")
        nc.sync.dma_start(ones_k[:], onesk_d[:])

        # scoped pools that die before the output projection
        p_wB = tc.alloc_tile_pool(name="wB", bufs=1)
        p_ck = tc.alloc_tile_pool(name="ck", bufs=1)
        p_tmp = tc.alloc_tile_pool(name="tmp", bufs=6)

        ckv = p_ck.tile([128, 4, S], BF16, tag="ckv")  # [dl%128, dl//128, s]
        cost = p_ck.tile([128, S], F32, tag="cos")
        sint = p_ck.tile([128, S], F32, tag="sin")
        wkvt = p_wB.tile([128, 4, 1024], BF16, tag="wkvt")

        # ============ Phase A: qT + c_kvT projection ============
        # 4+4 PSUM bank rotation. Per s-quarter: q groups then c_kv groups;
        # each sub-loop's evacuation overlaps the other's matmuls.
        with (
            tc.tile_pool(name="wA", bufs=1) as p_wA,
            tc.tile_pool(name="xts", bufs=12) as p_xt,
            tc.tile_pool(name="psAq", bufs=4, space="PSUM") as ps_aq,
            tc.tile_pool(name="psAc", bufs=4, space="PSUM") as ps_ac,
        ):
            wqct = p_wA.tile([128, 16, 1024], BF16, tag="wA")
            # first weight chunks lead the queue: the very first LDWEIGHTS
            # waits only on wqct k=0, so everything else goes after it
            for k in range(4):
                nc.sync.dma_start(wqct[:, k, :], wqct_d[:, k, :])
            nc.sync.dma_start(masks[:], mask_d[:])
            for k in range(4, 16):
                nc.sync.dma_start(wqct[:, k, :], wqct_d[:, k, :])
            for i in range(4):
                nc.sync.dma_start(wkvt[:, i, :], wkvt_d[:, i, :])

            def ckv_subloop(q, sq):
                caccs = [
                    ps_ac.tile([128, 512], F32, tag="pac", name=f"pac{m}")
                    for m in range(4)
                ]
                for k in range(16):
                    xt = p_xt.tile([128, 512], BF16, tag="xt")
                    nc.scalar.dma_start(xt[:], xt_d[k * 128 : (k + 1) * 128, sq])
                    for i in range(4):
                        nc.tensor.matmul(
                            caccs[i][:],
                            wqct[:, k, 512 + i * 128 : 640 + i * 128],
                            xt[:],
                            start=(k == 0),
                            stop=(k == 15),
                        )
                for i in range(4):
                    nc.scalar.copy(ckv[:, i, sq], caccs[i][:])

            def q_subloop(q, sq):
                qaccs = [
                    ps_aq.tile([128, 512], F32, tag="paq", name=f"pq{q}{m}")
                    for m in range(4)
                ]
                for k in range(16):
                    xt = p_xt.tile([128, 512], BF16, tag="xt")
                    nc.gpsimd.dma_start(xt[:], xt_d[k * 128 : (k + 1) * 128, sq])
                    for i in range(4):
                        nc.tensor.matmul(
                            qaccs[i][:],
                            wqct[:, k, i * 128 : (i + 1) * 128],
                            xt[:],
                            start=(k == 0),
                            stop=(k == 15),
                        )
                for i in range(4):
                    _rope_evac(
                        nc,
                        p_tmp,
                        qaccs[i][:],
                        qt[:, i, sq],
                        cost[:, sq],
                        sint[:, sq],
                    )

            for q in range(4):  # s-quarter
                sq = slice(q * 512, (q + 1) * 512)
                nc.sync.dma_start(cost[:, sq], cos_d[:, sq])
                nc.sync.dma_start(sint[:, sq], sin_d[:, sq])
                q_subloop(q, sq)
                ckv_subloop(q, sq)

        # ============ Merged B+C: kT/v decompression interleaved with
        # attention (scoresT / softmax / PV) ============
        p_kv = ent(tc.tile_pool(name="kv", bufs=1, side="right"))
        kt = p_kv.tile([128, HL, S], BF16, tag="kt")  # [dh, h, s]
        v = p_kv.tile([128, 16, HF], BF16, tag="v")  # [s%128, s//128, hf]

        # first half of WoT loads during the attention region (fresh SBUF,
        # no WAR wait) so the output projection starts without a weight stall
        p_wDe = ent(tc.tile_pool(name="wDe", bufs=1, side="right"))
        wde = p_wDe.tile([128, 12, 512], BF16, tag="wde")
        for m in range(12):
            nc.sync.dma_start(wde[:, m, :], wot_d[:, m, :])

        p_ex = tc.alloc_tile_pool(name="ex", bufs=8)
        p_acc = tc.alloc_tile_pool(name="exacc", bufs=4)
        p_bc = tc.alloc_tile_pool(name="bc", bufs=2)
        p_osb = tc.alloc_tile_pool(name="osb", bufs=2)
        p_oei = tc.alloc_tile_pool(name="oei", bufs=2)
        ps_bkv = tc.alloc_tile_pool(name="psBKV", bufs=1, space="PSUM")
        ps_s = tc.alloc_tile_pool(name="psS", bufs=2, space="PSUM")
        ps_o = tc.alloc_tile_pool(name="psO", bufs=2, space="PSUM")
        ps_n = tc.alloc_tile_pool(name="psN", bufs=1, space="PSUM")

        def kt_group(g):
            m, n = divmod(g, 4)
            sn = slice(n * 512, (n + 1) * 512)
            acc = ps_bkv.tile([128, 512], F32, tag="pkv", name=f"pk{g}")
            for i in range(4):
                nc.tensor.matmul(
                    acc[:],
                    wkvt[:, i, m * 128 : (m + 1) * 128],
                    ckv[:, i, sn],
                    start=(i == 0),
                    stop=(i == 3),
                )
            _rope_evac(nc, p_tmp, acc[:], kt[:, m, sn], cost[:, sn], sint[:, sn])

        def v_group(g):
            acc2 = ps_bkv.tile([128, 512], F32, tag="pkv", name=f"pv{g}")
            for i in range(4):
                nc.tensor.matmul(
                    acc2[:],
                    ckv[:, i, g * 128 : (g + 1) * 128],
                    wkvt[:, i, 512:1024],
                    start=(i == 0),
                    stop=(i == 3),
                )
            nc.scalar.copy(v[:, g, :], acc2[:])

        # decompression work interleaved after attention block (h, t); kt
        # group g feeds block (g//4, g%4), v group g feeds PV column j=g.
        interleave = {
            (0, 0): [("kt", 1), ("v", 4), ("v", 5), ("v", 6), ("v", 7)],
            (0, 1): [("kt", 2), ("v", 8), ("v", 9), ("v", 10), ("v", 11)],
            (0, 2): [("kt", 3), ("v", 12), ("v", 13), ("v", 14), ("v", 15)],
            (0, 3): [("kt", 4), ("kt", 5)],
            (1, 0): [("kt", 6)],
            (1, 1): [("kt", 7)],
            (1, 2): [("kt", 8)],
            (1, 3): [("kt", 9)],
            (2, 0): [("kt", 10)],
            (2, 1): [("kt", 11)],
            (2, 2): [("kt", 12), ("kt", 13)],
            (2, 3): [("kt", 14), ("kt", 15)],
        }

        def v_group_ps(g):
            acc2 = ps_s.tile([128, 1024], F32, tag="s", name=f"pvs{g}")
            for i in range(4):
                nc.tensor.matmul(
                    acc2[:, 0:512],
                    ckv[:, i, g * 128 : (g + 1) * 128],
                    wkvt[:, i, 512:1024],
                    start=(i == 0),
                    stop=(i == 3),
                )
            nc.vector.tensor_copy(v[:, g, :], acc2[:, 0:512])

        # prologue: first key block of head 0 + v chunks for its PV
        kt_group(0)
        v_group(0)
        v_group_ps(1)
        v_group_ps(2)
        v_group(3)

        # One continuous software pipeline across all (h, t) blocks:
        # scores/exp/mask run 2 pairs ahead of sums/pv, including over
        # block boundaries, so PE never drains waiting on ACT+DVE.
        blocks = [(h, t) for h in range(HL) for t in range(4)]
        ctxs = {}
        pend = []

        def emit_sums_pv(bi, jj, ex, lay):
            h, t, acc_o, exacc = ctxs[bi]
            nj = 4 * (t + 1)
            for j, cs, w, qo in lay:
                exsl = ex[:, cs : cs + w]
                # softmax denominator: accumulate exp tiles on DVE (bf16
                # packed = 2x mode); the partition reduce happens once per
                # block in a single 512-row ones-matmul.
                if j == 0:
                    nc.vector.tensor_copy(exacc[:], exsl)
                else:
                    nc.vector.tensor_add(
                        exacc[:, qo:512], exacc[:, qo:512], exsl
                    )
                nc.tensor.matmul(
                    acc_o[:, qo:512],
                    v[:, j, h * 128 : (h + 1) * 128],
                    exsl,
                    start=(j == 0), stop=(j == nj - 1),
                )
            if jj == nj - 2:
                _normalize(bi)

        def d_block_early(m):
            # output projection (n=0 s-tile) emitted inside head 3; qt
            # [:, :, 0:512] is final once every head's t=0 block normalized
            acc = ps_bkv.tile([128, 512], F32, tag="pkv", name=f"pde{m}")
            for i in range(4):
                nc.tensor.matmul(
                    acc[:],
                    wde[:, m, i * 128 : (i + 1) * 128],
                    qt[:, i, 0:512],
                    start=(i == 0),
                    stop=(i == 3),
                )
            o = p_oei.tile([128, 512], F32, tag="oei", name=f"oei{m}")
            if m % 2 == 0:
                nc.scalar.copy(o[:], acc[:])
            else:
                nc.vector.tensor_copy(o[:], acc[:])
            nc.sync.dma_start(out_d[m * 128 : (m + 1) * 128, 0:512], o[:])

        def _normalize(bi):
            h, t, acc_o, exacc = ctxs.pop(bi)
            sq = slice(t * 512, (t + 1) * 512)
            ou = p_osb.tile([128, 512], F32, tag="ou", name=f"ou{bi}")
            nc.scalar.copy(ou[:], acc_o[:])
            # softmax denominator: all-ones matmul broadcast-sums exacc
            # across partitions into a scratch bank (PE depends only on the
            # last exacc add), then a fast approx reciprocal + multiply on
            # DVE finish the normalize off the PE critical path.
            nb = ps_n.tile([128, 512], F32, tag="nb", name=f"nb{bi}")
            nc.tensor.matmul(nb[:, :], ones_k[:], exacc[:], start=True, stop=True)
            rbb = p_bc.tile([128, 512], F32, tag="rb", name=f"rb{bi}")
            nc.vector.reciprocal_approx_fast(rbb[:], nb[:, :])
            nc.vector.tensor_mul(qt[:, h, sq], ou[:], rbb[:])

        def chunk_width(di):
            # diagonal chunks only compute the live (unmasked) query range:
            # chunk di covers keys >= 128*di of its quarter, so queries below
            # 128*di are fully masked and skipped (bf16 matmuls are full-rate
            # at any width).
            if di < 1:
                return 512
            return (384, 256, 128)[di - 1]

        for bi, (h, t) in enumerate(blocks):
            nj = 4 * (t + 1)
            ctxs[bi] = (
                h,
                t,
                ps_o.tile([128, 512], F32, tag="o", name=f"o{bi}"),
                p_acc.tile([128, 512], BF16, tag="exacc", name=f"exacc{bi}"),
            )
            for jj in range(0, nj, 2):
                di0 = jj - 4 * t
                if di0 == 2:
                    # narrow diag pair (256+128 cols) fits the normalize
                    # bank, which idles between per-block normalizes
                    ps = ps_n.tile([128, 512], F32, tag="nb", name=f"sd{bi}")
                else:
                    ps = ps_s.tile([128, 1024], F32, tag="s", name=f"s{bi}_{jj}")
                lay = []
                cs = 0
                for d_ in range(2):
                    j = jj + d_
                    w = chunk_width(di0 + d_)
                    qo = 512 - w
                    nc.tensor.matmul(
                        ps[:, cs : cs + w],
                        kt[:, h, j * 128 : (j + 1) * 128],
                        qt[:, h, t * 512 + qo : (t + 1) * 512],
                        start=True,
                        stop=True,
                    )
                    lay.append((j, cs, w, qo))
                    cs += w
                ex = p_ex.tile([128, 1024], BF16, tag="ex", name=f"ex{bi}_{jj}")
                nc.scalar.activation(ex[:, :cs], ps[:, :cs], EXP, scale=SCALE)
                if di0 >= 0:  # diagonal pair -> causal mask (DVE, bf16 2x)
                    mb = 0 if di0 == 0 else 896
                    nc.vector.tensor_mul(
                        ex[:, :cs], ex[:, :cs], masks[:, mb : mb + cs]
                    )
                pend.append((bi, jj, ex, lay))
                if len(pend) > 2:
                    emit_sums_pv(*pend.pop(0))
            for kind, g in interleave.get((h, t), []):
                kt_group(g) if kind == "kt" else v_group(g)
            if (h, t) in ((3, 1), (3, 2), (3, 3)):
                for m in range(4 * (t - 1), 4 * t):
                    d_block_early(m)
        while pend:
            emit_sums_pv(*pend.pop(0))

        # free decompression-scope SBUF (LIFO), then preload WoT
        ps_n.release()
        ps_o.release()
        ps_s.release()
        ps_bkv.release()
        p_oei.release()
        p_osb.release()
        p_bc.release()
        p_acc.release()
        p_ex.release()
        p_tmp.release()
        p_ck.release()
        p_wB.release()

        # ================= Phase D: output projection =================
        # n-outer so the first s-tile only waits on the last head's t=0
        # normalize; PSUM evacuation alternates ACT/DVE; DMA from SP queue.
        with (
            tc.tile_pool(name="wD", bufs=1) as p_wD,
            tc.tile_pool(name="oev", bufs=6) as p_oev,
            tc.tile_pool(name="psD", bufs=4, space="PSUM") as ps_d,
        ):
            wot = p_wD.tile([128, 4, 512], BF16, tag="wot")
            for m in range(4):
                nc.sync.dma_start(wot[:, m, :], wot_d[:, m + 12, :])
            for n in range(4):  # s tile
                sn = slice(n * 512, (n + 1) * 512)
                for m in range(16):  # dout chunk
                    if n == 0 and m < 12:
                        continue  # emitted early inside head 3
                    acc = ps_d.tile([128, 512], F32, tag="pd")
                    wsrc = wde[:, m, :] if m < 12 else wot[:, m - 12, :]
                    for i in range(4):
                        nc.tensor.matmul(
                            acc[:],
                            wsrc[:, i * 128 : (i + 1) * 128],
                            qt[:, i, sn],
                            start=(i == 0),
                            stop=(i == 3),
                        )
                    o = p_oev.tile([128, 512], F32, tag="oev")
                    if m % 2 == 0:
                        nc.scalar.copy(o[:], acc[:])
                    else:
                        nc.vector.tensor_copy(o[:], acc[:])
                    nc.sync.dma_start(out_d[m * 128 : (m + 1) * 128, sn], o[:])


def _rope_tables():
    inv = 1.0 / (10000.0 ** (np.arange(0, DH, 2, dtype=np.float32) / DH))
    t = np.arange(S, dtype=np.float32)
    freqs = np.outer(t, inv)  # (S, 64)
    emb = np.concatenate([freqs, freqs], axis=-1)  # (S, 128)
    sinm = np.sin(emb).astype(np.float32)
    sinm[:, :64] *= -1.0  # sign-folded for the rotate-half add
    return (
        np.ascontiguousarray(np.cos(emb).T.astype(np.float32)),
        np.ascontiguousarray(sinm.T),
    )


def _masks():
    p = np.arange(128)[:, None]
    segs = []
    for di, w in [(0, 512), (1, 384), (2, 256), (3, 128)]:
        q = np.arange(512 - w, 512)[None, :]
        segs.append((p <= q - 128 * di).astype(np.float32))
    return np.ascontiguousarray(np.concatenate(segs, axis=1))


def kernel(hidden_states, Wq, Wc, Wk, Wv, Wo, _trace=False):
    hidden_states = np.asarray(hidden_states, dtype=np.float32)
    Wq = np.asarray(Wq, dtype=np.float32)
    Wc = np.asarray(Wc, dtype=np.float32)
    Wk = np.asarray(Wk, dtype=np.float32)
    Wv = np.asarray(Wv, dtype=np.float32)
    Wo = np.asarray(Wo, dtype=np.float32)

    if "nc" not in _CACHED:
        _CACHED["nc"] = _build()
    nc = _CACHED["nc"]

    import ml_dtypes

    bf16 = ml_dtypes.bfloat16
    cost, sint = _rope_tables()
    masks = _masks().astype(bf16)
    wct = np.ascontiguousarray(Wc.T)  # (D, DL)
    xts = [np.ascontiguousarray(hidden_states[b].T).astype(bf16) for b in range(B)]

    in_maps = []
    for c in range(NCORES):
        b, hg = divmod(c, HL)
        hs = hg * HF
        wq = Wq[hs : hs + HF].T  # (D, HF)
        wqct = np.concatenate([wq, wct], axis=1)  # (D, 1024)
        wqct = np.ascontiguousarray(wqct.reshape(16, 128, 1024).transpose(1, 0, 2)).astype(bf16)
        wk = Wk[hs : hs + HF].T  # (DL, HF)
        wv = Wv[hs : hs + HF].T
        wkvt = np.concatenate([wk, wv], axis=1)  # (DL, 1024)
        wkvt = np.ascontiguousarray(wkvt.reshape(4, 128, 1024).transpose(1, 0, 2)).astype(bf16)
        wot = Wo[:, hs : hs + HF].T  # (HF, D)
        wot = np.ascontiguousarray(
            wot.reshape(4, 128, 16, 128).transpose(1, 2, 0, 3).reshape(128, 16, 512)
        ).astype(bf16)
        in_maps.append(
            {
                "xt": xts[b],
                "wqct": wqct,
                "wkvt": wkvt,
                "wot": wot,
                "cost": cost,
                "sint": sint,
                "masks": masks,
                "onesk": np.ones((128, 128), bf16),
            }
        )

    res = run_bass_kernel_spmd(
        nc, in_maps, core_ids=list(range(NCORES)), trace=_trace
    )
    _CACHED["last_results"] = res

    out = np.empty((B, S, D), np.float32)
    for b in range(B):
        acc = res.results[4 * b]["outt"].astype(np.float32)
        for c in range(4 * b + 1, 4 * b + 4):
            acc = acc + res.results[c]["outt"]
        out[b] = acc.T
    return out

